# revision 26
# baseline (speedup 1.0000x reference)
"""MoE layer (top-2 of 8 experts, SwiGLU) on 8 Trainium2 NeuronCores.

Strategy (expert-parallel):
  - Router runs on host (fp64 selection to be robust, fp32 values to match
    the fp32 reference numerics). Tokens are gathered per expert on host.
  - Each of the 8 cores holds one expert's weights and runs the SwiGLU FFN
    over its gathered tokens:  y = cw * ((silu(x@w1) * (x@w3)) @ w2).
    The combine-weight multiply happens on device; the host only
    scatter-adds the two expert contributions per token (matching the
    reference's per-expert f32 accumulation order).
  - aux_loss is a cheap scalar computed on host from routing statistics.

The device kernel computes hT = w1T-chunk.T @ xT tiles directly in the
(H on partitions, tokens on free) orientation so the second matmul needs
no transpose. All matmul contraction runs in PSUM fp32.
"""

import os

import numpy as np
import ml_dtypes

os.environ.setdefault("MYCRO_LOCAL_CACHE", "1")

N_EXPERTS = 8
N_ACTIVE = 2
N_CORES = 8
TB = 512  # token block (matmul moving free dim)

# matmul precision mode: "bf16" | "f32r" | "f32"
# f32r: full-speed PE matmul over fp32 data with reduced internal multiply
# precision — measured ~2.6e-4 absmax error vs the fp32 reference (16x
# better than bf16) at ~15% more time than bf16.
MODE = os.environ.get("MOE_KERNEL_MODE", "f32r")

_BUILD_CACHE = {}


def _build(cap, D, H, mode):
    """Build the SPMD Bass program for one expert-core: inputs
    xT (D, cap), w1 (D, H), w3 (D, H), w2 (H, D), cw (128, cap/128);
    output y (cap, D) f32, already scaled by combine weights."""
    key = (cap, D, H, mode)
    if key in _BUILD_CACHE:
        return _BUILD_CACHE[key]

    from contextlib import ExitStack

    import concourse.bacc as bacc
    import concourse.mybir as mybir
    import concourse.tile as tile

    f32 = mybir.dt.float32
    if mode == "bf16":
        wdt = mybir.dt.bfloat16
    elif mode == "f32r":
        # FP32r: full-speed PE matmul on fp32 bits with reduced internal
        # precision. The BIR verifier requires every matmul operand's
        # producer to have float32r output dtype, so the DRAM params and
        # all matmul-feeding tiles are declared float32r end-to-end.
        wdt = mybir.dt.float32r
    else:
        wdt = f32

    def mm_ap(ap):
        return ap

    ND = D // 128
    NH = H // 128
    NMB = cap // 128
    ND512 = D // 512

    blocks = []
    t = 0
    while t < cap:
        b = min(TB, cap - t)
        blocks.append((t, b))
        t += b
    # Remainder block SECOND: the first block must be full-width so the
    # interleaved w2 residency load gets a long phase-A window, and the
    # remainder's full-size w1/w3 re-stream (22MB vs only ~40us of PE work
    # — DMA-bound on its own) then overlaps block 0's DMA-quiet phase B
    # instead of stalling the kernel ramp (remainder-first) or the drain
    # (remainder-last).
    if len(blocks) > 1 and blocks[-1][1] < TB:
        blocks = [blocks[0], blocks[-1]] + blocks[1:-1]

    # Bacc (not raw Bass): its compile() runs move_matmul_waits_to_ldweights
    # + generate_event_semaphores, which split multi-waits down to the
    # TRN2 per-instruction limit (walrus rejects >1 wait on e.g. TT).
    nc = bacc.Bacc()
    xT = nc.declare_dram_parameter("xT", [D, cap], wdt, isOutput=False)
    w1 = nc.declare_dram_parameter("w1", [D, H], wdt, isOutput=False)
    w3 = nc.declare_dram_parameter("w3", [D, H], wdt, isOutput=False)
    w2 = nc.declare_dram_parameter("w2", [H, D], wdt, isOutput=False)
    cw = nc.declare_dram_parameter("cw", [128, NMB], f32, isOutput=False)
    y = nc.declare_dram_parameter("y", [cap, D], f32, isOutput=True)

    xT_r = xT[:].rearrange("(nd p) t -> p nd t", p=128)
    w1_r = w1[:].rearrange("(nd p) h -> p nd h", p=128)
    w3_r = w3[:].rearrange("(nd p) h -> p nd h", p=128)
    w2_r = w2[:].rearrange("(nh p) d -> p nh d", p=128)
    y_r = y[:].rearrange("(b p) d -> b p d", p=128)

    with ExitStack() as ctx:
        tc = ctx.enter_context(tile.TileContext(nc))
        singles = ctx.enter_context(tc.tile_pool(name="singles", bufs=1))
        xpool = ctx.enter_context(
            tc.tile_pool(name="xp", bufs=2 if mode == "bf16" else 1)
        )
        wpool = ctx.enter_context(tc.tile_pool(name="wp", bufs=3))
        gpool = ctx.enter_context(
            tc.tile_pool(name="gp", bufs=2 if mode == "bf16" else 1)
        )
        tpool = ctx.enter_context(tc.tile_pool(name="tp", bufs=3))
        opool = ctx.enter_context(tc.tile_pool(name="op", bufs=3))
        # 8 PSUM banks: 3 bufs each for the ps1/ps3 accumulators (6 banks,
        # absorbs ACT/DVE drain jitter in the dominant phase) + 2 for the
        # phase-B output ring. Measured best of the 2/3, 3/2, 3+2+3 splits.
        hps = ctx.enter_context(tc.tile_pool(name="hps", bufs=3, space="PSUM"))
        ypsp = ctx.enter_context(tc.tile_pool(name="ypsp", bufs=2, space="PSUM"))

        cw_t = singles.tile([128, NMB], f32)
        nc.sync.dma_start(cw_t[:], cw[:])
        # w2 stays resident: rhs chunks (128 h-rows, D) for the second
        # matmul. The loads are emitted after block 0's phase-A DMAs (below)
        # so they don't hog the DMA queues ahead of the first matmuls.
        w2res = singles.tile([128, NH * D], wdt)
        w2_loaded = False

        for t0, tbs in blocks:
            nmb_t = tbs // 128
            # per-d tiles so the first matmul waits on one DMA, not all ND
            xts = []
            for d in range(ND):
                xtd = xpool.tile([128, tbs], wdt, tag=f"xt{d}")
                nc.sync.dma_start(xtd[:], xT_r[:, d, t0 : t0 + tbs])
                xts.append(xtd)
            g = gpool.tile([128, NH * tbs], wdt, tag="g")
            for hc in range(NH):
                w1c = wpool.tile([128, ND * 128], wdt, tag="w1c")
                w3c = wpool.tile([128, ND * 128], wdt, tag="w3c")
                nc.sync.dma_start(
                    w1c[:].rearrange("p (nd h) -> p nd h", nd=ND),
                    w1_r[:, :, hc * 128 : (hc + 1) * 128],
                )
                nc.sync.dma_start(
                    w3c[:].rearrange("p (nd h) -> p nd h", nd=ND),
                    w3_r[:, :, hc * 128 : (hc + 1) * 128],
                )
                if not w2_loaded:
                    # spread the w2 residency load across phase A of the
                    # first block instead of one burst that stalls phase B
                    nc.sync.dma_start(
                        w2res[:, hc * D : (hc + 1) * D], w2_r[:, hc, :]
                    )
                ps1 = hps.tile([128, tbs], f32, tag="ps1")
                ps3 = hps.tile([128, tbs], f32, tag="ps3")
                for d in range(ND):
                    nc.tensor.matmul(
                        ps1[:],
                        mm_ap(w1c[:, d * 128 : (d + 1) * 128]),
                        mm_ap(xts[d][:]),
                        start=(d == 0),
                        stop=(d == ND - 1),
                    )
                for d in range(ND):
                    nc.tensor.matmul(
                        ps3[:],
                        mm_ap(w3c[:, d * 128 : (d + 1) * 128]),
                        mm_ap(xts[d][:]),
                        start=(d == 0),
                        stop=(d == ND - 1),
                    )
                sil = tpool.tile([128, tbs], f32, tag="sil")
                nc.scalar.activation(
                    sil[:], ps1[:], mybir.ActivationFunctionType.Silu
                )
                nc.vector.tensor_mul(
                    g[:, hc * tbs : (hc + 1) * tbs], sil[:], ps3[:]
                )
            w2_loaded = True
            for n in range(ND512):
                for m in range(nmb_t):
                    gmb = t0 // 128 + m
                    yp = ypsp.tile([128, 512], f32, tag="yp")
                    for hc in range(NH):
                        nc.tensor.matmul(
                            yp[:],
                            mm_ap(
                                g[:, hc * tbs + m * 128 : hc * tbs + (m + 1) * 128]
                            ),
                            mm_ap(w2res[:, hc * D + n * 512 : hc * D + (n + 1) * 512]),
                            start=(hc == 0),
                            stop=(hc == NH - 1),
                        )
                    osb = opool.tile([128, 512], f32, tag="osb")
                    nc.vector.tensor_scalar_mul(
                        osb[:], yp[:], cw_t[:, gmb : gmb + 1]
                    )
                    nc.sync.dma_start(y_r[gmb, :, n * 512 : (n + 1) * 512], osb[:])

    nc.compile()
    _BUILD_CACHE[key] = nc
    return nc


def _route(xf, Wr):
    """Host router. Selection in f64 (robust to accumulation-order jitter),
    probabilities/weights in f32 to track the f32 reference."""
    T, D = xf.shape
    E = Wr.shape[1]
    lg64 = xf.astype(np.float64) @ Wr.astype(np.float64)
    order = np.argsort(-lg64, axis=-1, kind="stable")
    top2 = order[:, :N_ACTIVE]

    lg32 = xf @ Wr
    m32 = lg32.max(-1, keepdims=True)
    e32 = np.exp(lg32 - m32)
    p32 = e32 / e32.sum(-1, keepdims=True)
    tv = np.take_along_axis(p32, top2, axis=-1)
    tw = tv / tv.sum(-1, keepdims=True)

    counts = np.bincount(top2.ravel(), minlength=E)
    f = counts.astype(np.float64) / (T * N_ACTIVE)
    P = p32.astype(np.float64).mean(axis=0)
    aux = np.float32(E * np.sum(f * P))
    return top2, tw, counts, aux


def _ensure_ntff_hook():
    """This image's ``antenv`` package lacks ``axon_hooks``; bass_utils
    imports it unconditionally when tracing is requested. Synthesize the
    module (with a working ctypes NTFF hook when the axon .so is present)
    so profiling works instead of crashing."""
    import sys
    import types

    try:
        import antenv.axon_hooks  # noqa: F401

        return
    except ImportError:
        pass
    hook = None
    try:
        from trn_agent_boot.trn_boot import _ntff_profile_via_ctypes

        so = "/opt/axon/libaxon_pjrt.so"
        if os.path.exists(so):
            hook = _ntff_profile_via_ctypes(so)
    except Exception:
        hook = None
    mod = types.ModuleType("antenv.axon_hooks")
    state = {"hook": hook}
    mod.set_axon_ntff_profile_hook = lambda h: state.__setitem__("hook", h)
    mod.get_axon_ntff_profile_hook = lambda: state["hook"]
    sys.modules["antenv.axon_hooks"] = mod
    try:
        import antenv

        antenv.axon_hooks = mod
    except Exception:
        pass


def _patch_upload():
    """Trace post-processing uploads artifacts to S3; make failures
    non-fatal (local profiling still works without the upload)."""
    try:
        import concourse.bass_utils as bu

        orig = bu.upload_artifacts
        if getattr(orig, "_moe_safe", False):
            return

        def safe(tmpdir):
            try:
                return orig(tmpdir)
            except Exception:
                return "local://" + tmpdir

        safe._moe_safe = True
        bu.upload_artifacts = safe
    except Exception:
        pass


LAST_RESULTS = None


def kernel(x, Wr, w1, w3, w2):
    global LAST_RESULTS
    x = np.asarray(x)
    Wr = np.asarray(Wr, dtype=np.float32)
    w1 = np.asarray(w1, dtype=np.float32)
    w3 = np.asarray(w3, dtype=np.float32)
    w2 = np.asarray(w2, dtype=np.float32)
    B, S, D = x.shape
    T = B * S
    E, _, H = w1.shape
    xf = np.ascontiguousarray(x.reshape(T, D), dtype=np.float32)

    top2, tw, counts, aux = _route(xf, Wr)

    cap = int(max(256, ((counts.max() + 127) // 128) * 128))
    NMB = cap // 128
    wdt = ml_dtypes.bfloat16 if MODE == "bf16" else np.float32

    nc = _build(cap, D, H, MODE)

    idx_per_e = []
    in_maps = []
    for e in range(E):
        hit = top2 == e
        idx = np.nonzero(hit.any(axis=1))[0]
        idx_per_e.append(idx)
        w_tok = np.where(hit[idx, 0], tw[idx, 0], tw[idx, 1]).astype(np.float32)
        cnt = len(idx)
        xT_e = np.zeros((D, cap), dtype=wdt)
        xT_e[:, :cnt] = xf[idx].T.astype(wdt)
        cwp = np.zeros(cap, dtype=np.float32)
        cwp[:cnt] = w_tok
        cw_t = np.ascontiguousarray(cwp.reshape(NMB, 128).T)
        in_maps.append(
            {
                "xT": xT_e,
                "w1": np.ascontiguousarray(w1[e]).astype(wdt),
                "w3": np.ascontiguousarray(w3[e]).astype(wdt),
                "w2": np.ascontiguousarray(w2[e]).astype(wdt),
                "cw": cw_t,
            }
        )

    _ensure_ntff_hook()
    _patch_upload()
    from concourse.bass_utils import run_bass_kernel_spmd

    try:
        res = run_bass_kernel_spmd(nc, in_maps, list(range(N_CORES)))
    except Exception:
        # The axon-tunneled devices occasionally fail an execution
        # spuriously; one retry (the NEFF is already compiled) recovers it.
        res = run_bass_kernel_spmd(nc, in_maps, list(range(N_CORES)))
    LAST_RESULTS = res

    out = np.zeros((T, D), dtype=np.float32)
    for e in range(E):
        idx = idx_per_e[e]
        out[idx] += res.results[e]["y"][: len(idx)]
    return out.reshape(B, S, D), aux


# revision 27
# speedup vs baseline: 1.0552x; 1.0552x over previous
"""MoE layer (top-2 of 8 experts, SwiGLU) on 8 Trainium2 NeuronCores.

Strategy (expert-parallel):
  - Router runs on host (fp64 selection to be robust, fp32 values to match
    the fp32 reference numerics). Tokens are gathered per expert on host.
  - Each of the 8 cores holds one expert's weights and runs the SwiGLU FFN
    over its gathered tokens:  y = cw * ((silu(x@w1) * (x@w3)) @ w2).
    The combine-weight multiply happens on device; the host only
    scatter-adds the two expert contributions per token (matching the
    reference's per-expert f32 accumulation order).
  - aux_loss is a cheap scalar computed on host from routing statistics.

The device kernel computes hT = w1T-chunk.T @ xT tiles directly in the
(H on partitions, tokens on free) orientation so the second matmul needs
no transpose. All matmul contraction runs in PSUM fp32.
"""

import os

import numpy as np
import ml_dtypes

os.environ.setdefault("MYCRO_LOCAL_CACHE", "1")

N_EXPERTS = 8
N_ACTIVE = 2
N_CORES = 8
TB = 512  # token block (matmul moving free dim)

# matmul precision mode: "bf16" | "f32r" | "f32"
# f32r: full-speed PE matmul over fp32 data with reduced internal multiply
# precision — measured ~2.6e-4 absmax error vs the fp32 reference (16x
# better than bf16) at ~15% more time than bf16.
MODE = os.environ.get("MOE_KERNEL_MODE", "f32r")

_BUILD_CACHE = {}


def _build(cap, D, H, mode):
    """Build the SPMD Bass program for one expert-core: inputs
    xT (D, cap), w1 (D, H), w3 (D, H), w2 (H, D), cw (128, cap/128);
    output y (cap, D) f32, already scaled by combine weights."""
    key = (cap, D, H, mode)
    if key in _BUILD_CACHE:
        return _BUILD_CACHE[key]

    from contextlib import ExitStack

    import concourse.bacc as bacc
    import concourse.mybir as mybir
    import concourse.tile as tile

    f32 = mybir.dt.float32
    if mode == "bf16":
        wdt = mybir.dt.bfloat16
    elif mode == "f32r":
        # FP32r: full-speed PE matmul on fp32 bits with reduced internal
        # precision. The BIR verifier requires every matmul operand's
        # producer to have float32r output dtype, so the DRAM params and
        # all matmul-feeding tiles are declared float32r end-to-end.
        wdt = mybir.dt.float32r
    else:
        wdt = f32

    def mm_ap(ap):
        return ap

    ND = D // 128
    NH = H // 128
    NMB = cap // 128
    ND512 = D // 512

    blocks = []
    t = 0
    while t < cap:
        b = min(TB, cap - t)
        blocks.append((t, b))
        t += b
    # Remainder block SECOND: the first block must be full-width so the
    # interleaved w2 residency load gets a long phase-A window, and the
    # remainder's full-size w1/w3 re-stream (22MB vs only ~40us of PE work
    # — DMA-bound on its own) then overlaps block 0's DMA-quiet phase B
    # instead of stalling the kernel ramp (remainder-first) or the drain
    # (remainder-last).
    if len(blocks) > 2 and blocks[-1][1] < TB:
        blocks = blocks[0:2] + [blocks[-1]] + blocks[2:-1]
    elif len(blocks) > 1 and blocks[-1][1] < TB:
        blocks = [blocks[0], blocks[-1]] + blocks[1:-1]

    # Bacc (not raw Bass): its compile() runs move_matmul_waits_to_ldweights
    # + generate_event_semaphores, which split multi-waits down to the
    # TRN2 per-instruction limit (walrus rejects >1 wait on e.g. TT).
    nc = bacc.Bacc()
    xT = nc.declare_dram_parameter("xT", [D, cap], wdt, isOutput=False)
    w1 = nc.declare_dram_parameter("w1", [D, H], wdt, isOutput=False)
    w3 = nc.declare_dram_parameter("w3", [D, H], wdt, isOutput=False)
    w2 = nc.declare_dram_parameter("w2", [H, D], wdt, isOutput=False)
    cw = nc.declare_dram_parameter("cw", [128, NMB], f32, isOutput=False)
    y = nc.declare_dram_parameter("y", [cap, D], f32, isOutput=True)

    xT_r = xT[:].rearrange("(nd p) t -> p nd t", p=128)
    w1_r = w1[:].rearrange("(nd p) h -> p nd h", p=128)
    w3_r = w3[:].rearrange("(nd p) h -> p nd h", p=128)
    w2_r = w2[:].rearrange("(nh p) d -> p nh d", p=128)
    y_r = y[:].rearrange("(b p) d -> b p d", p=128)

    with ExitStack() as ctx:
        tc = ctx.enter_context(tile.TileContext(nc))
        singles = ctx.enter_context(tc.tile_pool(name="singles", bufs=1))
        xpool = ctx.enter_context(
            tc.tile_pool(name="xp", bufs=2 if mode == "bf16" else 1)
        )
        wpool = ctx.enter_context(tc.tile_pool(name="wp", bufs=3))
        gpool = ctx.enter_context(
            tc.tile_pool(name="gp", bufs=2 if mode == "bf16" else 1)
        )
        tpool = ctx.enter_context(tc.tile_pool(name="tp", bufs=3))
        opool = ctx.enter_context(tc.tile_pool(name="op", bufs=3))
        # 8 PSUM banks: 3 bufs each for the ps1/ps3 accumulators (6 banks,
        # absorbs ACT/DVE drain jitter in the dominant phase) + 2 for the
        # phase-B output ring. Measured best of the 2/3, 3/2, 3+2+3 splits.
        hps = ctx.enter_context(tc.tile_pool(name="hps", bufs=3, space="PSUM"))
        ypsp = ctx.enter_context(tc.tile_pool(name="ypsp", bufs=2, space="PSUM"))

        cw_t = singles.tile([128, NMB], f32)
        nc.sync.dma_start(cw_t[:], cw[:])
        # w2 stays resident: rhs chunks (128 h-rows, D) for the second
        # matmul. The loads are emitted after block 0's phase-A DMAs (below)
        # so they don't hog the DMA queues ahead of the first matmuls.
        w2res = singles.tile([128, NH * D], wdt)
        w2_loaded = False

        for t0, tbs in blocks:
            nmb_t = tbs // 128
            # per-d tiles so the first matmul waits on one DMA, not all ND
            xts = []
            for d in range(ND):
                xtd = xpool.tile([128, tbs], wdt, tag=f"xt{d}")
                nc.sync.dma_start(xtd[:], xT_r[:, d, t0 : t0 + tbs])
                xts.append(xtd)
            g = gpool.tile([128, NH * tbs], wdt, tag="g")
            for hc in range(NH):
                w1c = wpool.tile([128, ND * 128], wdt, tag="w1c")
                w3c = wpool.tile([128, ND * 128], wdt, tag="w3c")
                nc.sync.dma_start(
                    w1c[:].rearrange("p (nd h) -> p nd h", nd=ND),
                    w1_r[:, :, hc * 128 : (hc + 1) * 128],
                )
                nc.sync.dma_start(
                    w3c[:].rearrange("p (nd h) -> p nd h", nd=ND),
                    w3_r[:, :, hc * 128 : (hc + 1) * 128],
                )
                if not w2_loaded:
                    # spread the w2 residency load across phase A of the
                    # first block instead of one burst that stalls phase B
                    nc.sync.dma_start(
                        w2res[:, hc * D : (hc + 1) * D], w2_r[:, hc, :]
                    )
                ps1 = hps.tile([128, tbs], f32, tag="ps1")
                ps3 = hps.tile([128, tbs], f32, tag="ps3")
                for d in range(ND):
                    nc.tensor.matmul(
                        ps1[:],
                        mm_ap(w1c[:, d * 128 : (d + 1) * 128]),
                        mm_ap(xts[d][:]),
                        start=(d == 0),
                        stop=(d == ND - 1),
                    )
                for d in range(ND):
                    nc.tensor.matmul(
                        ps3[:],
                        mm_ap(w3c[:, d * 128 : (d + 1) * 128]),
                        mm_ap(xts[d][:]),
                        start=(d == 0),
                        stop=(d == ND - 1),
                    )
                sil = tpool.tile([128, tbs], f32, tag="sil")
                nc.scalar.activation(
                    sil[:], ps1[:], mybir.ActivationFunctionType.Silu
                )
                nc.vector.tensor_mul(
                    g[:, hc * tbs : (hc + 1) * tbs], sil[:], ps3[:]
                )
            w2_loaded = True
            for n in range(ND512):
                for m in range(nmb_t):
                    gmb = t0 // 128 + m
                    yp = ypsp.tile([128, 512], f32, tag="yp")
                    for hc in range(NH):
                        nc.tensor.matmul(
                            yp[:],
                            mm_ap(
                                g[:, hc * tbs + m * 128 : hc * tbs + (m + 1) * 128]
                            ),
                            mm_ap(w2res[:, hc * D + n * 512 : hc * D + (n + 1) * 512]),
                            start=(hc == 0),
                            stop=(hc == NH - 1),
                        )
                    osb = opool.tile([128, 512], f32, tag="osb")
                    nc.vector.tensor_scalar_mul(
                        osb[:], yp[:], cw_t[:, gmb : gmb + 1]
                    )
                    nc.sync.dma_start(y_r[gmb, :, n * 512 : (n + 1) * 512], osb[:])

    nc.compile()
    _BUILD_CACHE[key] = nc
    return nc


def _route(xf, Wr):
    """Host router. Selection in f64 (robust to accumulation-order jitter),
    probabilities/weights in f32 to track the f32 reference."""
    T, D = xf.shape
    E = Wr.shape[1]
    lg64 = xf.astype(np.float64) @ Wr.astype(np.float64)
    order = np.argsort(-lg64, axis=-1, kind="stable")
    top2 = order[:, :N_ACTIVE]

    lg32 = xf @ Wr
    m32 = lg32.max(-1, keepdims=True)
    e32 = np.exp(lg32 - m32)
    p32 = e32 / e32.sum(-1, keepdims=True)
    tv = np.take_along_axis(p32, top2, axis=-1)
    tw = tv / tv.sum(-1, keepdims=True)

    counts = np.bincount(top2.ravel(), minlength=E)
    f = counts.astype(np.float64) / (T * N_ACTIVE)
    P = p32.astype(np.float64).mean(axis=0)
    aux = np.float32(E * np.sum(f * P))
    return top2, tw, counts, aux


def _ensure_ntff_hook():
    """This image's ``antenv`` package lacks ``axon_hooks``; bass_utils
    imports it unconditionally when tracing is requested. Synthesize the
    module (with a working ctypes NTFF hook when the axon .so is present)
    so profiling works instead of crashing."""
    import sys
    import types

    try:
        import antenv.axon_hooks  # noqa: F401

        return
    except ImportError:
        pass
    hook = None
    try:
        from trn_agent_boot.trn_boot import _ntff_profile_via_ctypes

        so = "/opt/axon/libaxon_pjrt.so"
        if os.path.exists(so):
            hook = _ntff_profile_via_ctypes(so)
    except Exception:
        hook = None
    mod = types.ModuleType("antenv.axon_hooks")
    state = {"hook": hook}
    mod.set_axon_ntff_profile_hook = lambda h: state.__setitem__("hook", h)
    mod.get_axon_ntff_profile_hook = lambda: state["hook"]
    sys.modules["antenv.axon_hooks"] = mod
    try:
        import antenv

        antenv.axon_hooks = mod
    except Exception:
        pass


def _patch_upload():
    """Trace post-processing uploads artifacts to S3; make failures
    non-fatal (local profiling still works without the upload)."""
    try:
        import concourse.bass_utils as bu

        orig = bu.upload_artifacts
        if getattr(orig, "_moe_safe", False):
            return

        def safe(tmpdir):
            try:
                return orig(tmpdir)
            except Exception:
                return "local://" + tmpdir

        safe._moe_safe = True
        bu.upload_artifacts = safe
    except Exception:
        pass


LAST_RESULTS = None


def kernel(x, Wr, w1, w3, w2):
    global LAST_RESULTS
    x = np.asarray(x)
    Wr = np.asarray(Wr, dtype=np.float32)
    w1 = np.asarray(w1, dtype=np.float32)
    w3 = np.asarray(w3, dtype=np.float32)
    w2 = np.asarray(w2, dtype=np.float32)
    B, S, D = x.shape
    T = B * S
    E, _, H = w1.shape
    xf = np.ascontiguousarray(x.reshape(T, D), dtype=np.float32)

    top2, tw, counts, aux = _route(xf, Wr)

    cap = int(max(256, ((counts.max() + 127) // 128) * 128))
    NMB = cap // 128
    wdt = ml_dtypes.bfloat16 if MODE == "bf16" else np.float32

    nc = _build(cap, D, H, MODE)

    idx_per_e = []
    in_maps = []
    for e in range(E):
        hit = top2 == e
        idx = np.nonzero(hit.any(axis=1))[0]
        idx_per_e.append(idx)
        w_tok = np.where(hit[idx, 0], tw[idx, 0], tw[idx, 1]).astype(np.float32)
        cnt = len(idx)
        xT_e = np.zeros((D, cap), dtype=wdt)
        xT_e[:, :cnt] = xf[idx].T.astype(wdt)
        cwp = np.zeros(cap, dtype=np.float32)
        cwp[:cnt] = w_tok
        cw_t = np.ascontiguousarray(cwp.reshape(NMB, 128).T)
        in_maps.append(
            {
                "xT": xT_e,
                "w1": np.ascontiguousarray(w1[e]).astype(wdt),
                "w3": np.ascontiguousarray(w3[e]).astype(wdt),
                "w2": np.ascontiguousarray(w2[e]).astype(wdt),
                "cw": cw_t,
            }
        )

    _ensure_ntff_hook()
    _patch_upload()
    from concourse.bass_utils import run_bass_kernel_spmd

    try:
        res = run_bass_kernel_spmd(nc, in_maps, list(range(N_CORES)))
    except Exception:
        # The axon-tunneled devices occasionally fail an execution
        # spuriously; one retry (the NEFF is already compiled) recovers it.
        res = run_bass_kernel_spmd(nc, in_maps, list(range(N_CORES)))
    LAST_RESULTS = res

    out = np.zeros((T, D), dtype=np.float32)
    for e in range(E):
        idx = idx_per_e[e]
        out[idx] += res.results[e]["y"][: len(idx)]
    return out.reshape(B, S, D), aux
